# revision 1
# baseline (speedup 1.0000x reference)
"""BezierAlign Trainium2 kernel.

Full inputs -> full output. Shards the R=256 ROIs across 8 NeuronCores (32
ROIs/core); the feature map is replicated to every core in a "quad block"
layout (each 4KB block holds the 2x2 pixel footprint of a bilinear sample)
so one indirect-DMA descriptor fetches all 4 corners of one sample.

Per-core device program:
  1. Evaluate the 4 cubic Bezier curves per ROI on 32 partitions (roi-major),
     fold the +-0.25*bin sample offsets and the -0.5 align shift into shifted
     endpoint curves, PE-transpose them to pw-on-partition layout.
  2. Per ROI, compute sample coords / validity / bilinear weights / gather
     offsets for all 1024 bins x 4 samples with ~40 DVE ops (bins on
     partitions, f32 throughout; floor via round(x-0.5) into int32).
  3. Per 128-bin tile: 4 indirect gathers ([128,1024] f32 each), then 16
     diag(weight) fp32 matmuls accumulating into 2 PSUM tiles, which yields
     the output directly transposed to [C, bins]; copy to SBUF, DMA out.
"""

import numpy as np

# problem shapes (hardcoded per contract)
N, C, H, W = 2, 256, 160, 160
R = 256
OUT_H, OUT_W = 16, 64
SCALE = 0.25
NCORES = 8
K = R // NCORES          # 32 rois per core
NT = (OUT_H * OUT_W) // 128   # 8 tiles of 128 bins per roi
HW = H * W

import os
_CACHE = {}
# PE combine in float32r (4x faster matmuls; ~1.6e-4 vs ~3e-5 rel err)
USE_F32R = os.environ.get("BEZ_F32R", "1") == "1"


def _host_constants():
    f32 = np.float32
    u = (np.arange(OUT_W, dtype=f32) / f32(OUT_W)).astype(f32)
    mt = (f32(1.0) - u).astype(f32)
    basis = np.stack([mt**3, 3 * u * mt**2, 3 * u**2 * mt, u**3]).astype(f32)  # [4,64]
    basis32 = np.broadcast_to(basis.reshape(1, 4 * OUT_W), (K, 4 * OUT_W)).copy()
    p = np.arange(128)
    t = np.arange(NT)
    v8 = (((2 * t[None, :] + (p[:, None] >= 64)).astype(f32)) / f32(16.0)).astype(f32)
    return basis32, v8


def _build_feat4(x):
    """x [N, C, H, W] f32 -> [N*H*W, 4C]; block(n,y,x) = [f(y,x), f(y+1,x),
    f(y,x+1), f(y+1,x+1)] with out-of-image parts zeroed."""
    f = np.ascontiguousarray(x.transpose(0, 2, 3, 1))     # [N,H,W,C]
    fy = np.zeros_like(f)
    fy[:, :-1] = f[:, 1:]
    a = np.concatenate([f, fy], axis=-1)                  # [N,H,W,2C]
    ax = np.zeros_like(a)
    ax[:, :, :-1] = a[:, :, 1:]
    feat4 = np.concatenate([a, ax], axis=-1)              # [N,H,W,4C]
    return np.ascontiguousarray(feat4.reshape(N * HW, 4 * C))


def _ap_view(ap, dims):
    """View an AP with custom free dims [(stride, count), ...] (partition dim kept)."""
    import concourse.bass as bass
    return bass.AP(tensor=ap.tensor, offset=ap.offset,
                   ap=[list(ap.ap[0])] + [[s, c] for s, c in dims])


def _build_nc(nrep=1):
    from contextlib import ExitStack
    import concourse.bacc as bacc
    import concourse.bass as bass
    import concourse.tile as tile
    from concourse import mybir
    from concourse.masks import make_identity

    f32 = mybir.dt.float32
    i32 = mybir.dt.int32
    Alu = mybir.AluOpType

    f32r = mybir.dt.float32r
    gdt = f32r if USE_F32R else f32

    nc = bacc.Bacc(None, target_bir_lowering=False)

    feat4 = nc.dram_tensor("feat4", [N * HW, 4 * C], gdt, kind="ExternalInput")
    rois = nc.dram_tensor("rois", [K, 17], f32, kind="ExternalInput")
    basis = nc.dram_tensor("basis", [K, 4 * OUT_W], f32, kind="ExternalInput")
    v8c = nc.dram_tensor("v8c", [128, NT], f32, kind="ExternalInput")
    out = nc.dram_tensor("out", [K, C, OUT_H, OUT_W], f32, kind="ExternalOutput")
    # [K, C, 1024] -> (k, h, p, t, b): c = h*128 + p, bin = t*128 + b
    out_v = out.rearrange("k (h p) (t c) w -> k p h t (c w)", h=2, c=2)

    with tile.TileContext(nc) as tc, ExitStack() as ctx:
        singles = ctx.enter_context(tc.tile_pool(name="singles", bufs=1))
        scratch = ctx.enter_context(tc.tile_pool(name="scratch", bufs=2))
        tabs = ctx.enter_context(tc.tile_pool(name="tabs", bufs=3))
        gpool = ctx.enter_context(tc.tile_pool(name="gpool", bufs=4))
        dpool = ctx.enter_context(tc.tile_pool(name="dpool", bufs=8))
        spool = ctx.enter_context(tc.tile_pool(name="spool", bufs=4))
        pp_t = ctx.enter_context(tc.tile_pool(name="pp_t", bufs=1, space="PSUM"))
        pp_mm = ctx.enter_context(tc.tile_pool(name="pp_mm", bufs=3, space="PSUM"))
        pp_tr = ctx.enter_context(tc.tile_pool(name="pp_tr", bufs=2, space="PSUM"))

        ident = singles.tile([128, 128], f32)
        make_identity(nc, ident[:])
        v8_t = singles.tile([128, NT], f32)
        nc.sync.dma_start(out=v8_t[:], in_=v8c[:])
        r_t = singles.tile([K, 17], f32)
        nc.sync.dma_start(out=r_t[:], in_=rois[:])
        b_t = singles.tile([K, 4, OUT_W], f32)
        nc.sync.dma_start(out=b_t[:], in_=basis[:].rearrange("k (a u) -> k a u", a=4))

        # control points: px = rois[:, 1::2]*0.25, py = rois[:, 2::2]*0.25
        px = scratch.tile([K, 8], f32, tag="px")
        py = scratch.tile([K, 8], f32, tag="py")
        r_ap = r_t[:]
        px_src = bass.AP(tensor=r_ap.tensor, offset=r_ap.offset + 1, ap=[list(r_ap.ap[0]), [2, 8]])
        py_src = bass.AP(tensor=r_ap.tensor, offset=r_ap.offset + 2, ap=[list(r_ap.ap[0]), [2, 8]])
        nc.vector.tensor_scalar(out=px[:], in0=px_src, scalar1=SCALE, scalar2=None, op0=Alu.mult)
        nc.vector.tensor_scalar(out=py[:], in0=py_src, scalar1=SCALE, scalar2=None, op0=Alu.mult)

        # curves [K, 64]: cv = sum_a B[a] * p[a(+4)]
        def bezier(dst, ptile, o):
            acc = scratch.tile([K, OUT_W], f32, tag="bzacc")
            tmp = scratch.tile([K, OUT_W], f32, tag="bztmp")
            nc.vector.tensor_scalar(out=acc[:], in0=b_t[:, 0, :], scalar1=ptile[:, o:o+1],
                                    scalar2=None, op0=Alu.mult)
            for a in (1, 2, 3):
                nc.vector.tensor_scalar(out=tmp[:], in0=b_t[:, a, :], scalar1=ptile[:, o+a:o+a+1],
                                        scalar2=None, op0=Alu.mult)
                nc.vector.tensor_tensor(out=dst[:] if a == 3 else acc[:],
                                        in0=acc[:], in1=tmp[:], op=Alu.add)

        x0 = scratch.tile([K, OUT_W], f32, tag="x0"); bezier(x0, px, 0)
        x1 = scratch.tile([K, OUT_W], f32, tag="x1"); bezier(x1, px, 4)
        y0 = scratch.tile([K, OUT_W], f32, tag="y0"); bezier(y0, py, 0)
        y1 = scratch.tile([K, OUT_W], f32, tag="y1"); bezier(y1, py, 4)

        # roi_w/h -> bwq = roi_w*0.25/64, bhq = roi_h*0.25/16  [K,1]
        def quarter_bin(ptile, scale_imm, tag):
            d1 = scratch.tile([K, 1], f32, tag=tag + "d1")
            d2 = scratch.tile([K, 1], f32, tag=tag + "d2")
            dn = scratch.tile([K, 1], f32, tag=tag + "dn")
            q = scratch.tile([K, 1], f32, tag=tag)
            nc.vector.tensor_tensor(out=d1[:], in0=ptile[:, 0:1], in1=ptile[:, 3:4], op=Alu.subtract)
            nc.vector.tensor_scalar(out=dn[:], in0=d1[:], scalar1=-1.0, scalar2=None, op0=Alu.mult)
            nc.vector.tensor_tensor(out=d1[:], in0=d1[:], in1=dn[:], op=Alu.max)
            nc.vector.tensor_tensor(out=d2[:], in0=ptile[:, 4:5], in1=ptile[:, 7:8], op=Alu.subtract)
            nc.vector.tensor_scalar(out=dn[:], in0=d2[:], scalar1=-1.0, scalar2=None, op0=Alu.mult)
            nc.vector.tensor_tensor(out=d2[:], in0=d2[:], in1=dn[:], op=Alu.max)
            nc.vector.tensor_tensor(out=d1[:], in0=d1[:], in1=d2[:], op=Alu.max)
            nc.vector.tensor_scalar(out=q[:], in0=d1[:], scalar1=scale_imm, scalar2=None, op0=Alu.mult)
            return q

        bwq = quarter_bin(px, 0.25 / OUT_W, "bwq")
        bhq = quarter_bin(py, 0.25 / OUT_H, "bhq")

        # 9 shifted curves [K, 64]: order xm0 xm1 xp0 xp1 ym0 ym1 yp0 yp1 base
        curves = scratch.tile([K, 9, OUT_W], f32, tag="curves")
        spec = [(x0, bwq, Alu.subtract, 0), (x1, bwq, Alu.subtract, 1),
                (x0, bwq, Alu.add, 2), (x1, bwq, Alu.add, 3),
                (y0, bhq, Alu.subtract, 4), (y1, bhq, Alu.subtract, 5),
                (y0, bhq, Alu.add, 6), (y1, bhq, Alu.add, 7)]
        for cv, qq, op, idx in spec:
            nc.vector.tensor_scalar(out=curves[:, idx, :], in0=cv[:], scalar1=qq[:, 0:1],
                                    scalar2=0.5, op0=op, op1=Alu.subtract)
        # base = batch * HW broadcast along 64
        base_c = scratch.tile([K, 1], f32, tag="base_c")
        nc.vector.tensor_scalar(out=base_c[:], in0=r_t[:, 0:1], scalar1=float(HW),
                                scalar2=None, op0=Alu.mult)
        bc_ap = base_c[:]
        nc.vector.tensor_scalar(
            out=curves[:, 8, :],
            in0=bass.AP(tensor=bc_ap.tensor, offset=bc_ap.offset, ap=[list(bc_ap.ap[0]), [0, OUT_W]]),
            scalar1=0.0, scalar2=None, op0=Alu.add)

        # transpose to TT [128, 9, K]: TT[p, q, r] = curves[r, q, p % 64]
        TT = singles.tile([128, 9, K], f32)
        for q in range(9):
            ps = pp_t.tile([128, K], f32, tag="tps", space="PSUM")
            cdup = scratch.tile([K, 128], f32, tag="cdup")
            cin = curves[:, q, :]
            dup = bass.AP(tensor=cin.tensor, offset=cin.offset,
                          ap=[list(cin.ap[0]), [0, 2], list(cin.ap[-1])])
            nc.vector.tensor_copy(out=cdup[:], in_=dup)
            nc.tensor.transpose(out=ps[:], in_=cdup[:], identity=ident[:K, :K])
            nc.vector.tensor_copy(out=TT[:, q, :], in_=ps[:])

        def ttcol(q, r):
            return TT[:, q, r:r+1]

        IY, IX, T8 = 2, 2, NT

        def main_work():
         for r in range(K):
            # deltas [128,1]
            dxm = tabs.tile([128, 1], f32, tag="dxm")
            dxp = tabs.tile([128, 1], f32, tag="dxp")
            dym = tabs.tile([128, 1], f32, tag="dym")
            dyp = tabs.tile([128, 1], f32, tag="dyp")
            nc.vector.tensor_tensor(out=dxm[:], in0=ttcol(1, r), in1=ttcol(0, r), op=Alu.subtract)
            nc.vector.tensor_tensor(out=dxp[:], in0=ttcol(3, r), in1=ttcol(2, r), op=Alu.subtract)
            nc.vector.tensor_tensor(out=dym[:], in0=ttcol(5, r), in1=ttcol(4, r), op=Alu.subtract)
            nc.vector.tensor_tensor(out=dyp[:], in0=ttcol(7, r), in1=ttcol(6, r), op=Alu.subtract)

            # XX [128, 2(ix), 8(t)] = x0S + V8*dx ; YY [128, 2(iy), 8]
            XX = tabs.tile([128, IX, T8], f32, tag="XX")
            YY = tabs.tile([128, IY, T8], f32, tag="YY")
            nc.vector.tensor_scalar(out=XX[:, 0, :], in0=v8_t[:], scalar1=dxm[:, 0:1],
                                    scalar2=ttcol(0, r), op0=Alu.mult, op1=Alu.add)
            nc.vector.tensor_scalar(out=XX[:, 1, :], in0=v8_t[:], scalar1=dxp[:, 0:1],
                                    scalar2=ttcol(2, r), op0=Alu.mult, op1=Alu.add)
            nc.vector.tensor_scalar(out=YY[:, 0, :], in0=v8_t[:], scalar1=dym[:, 0:1],
                                    scalar2=ttcol(4, r), op0=Alu.mult, op1=Alu.add)
            nc.vector.tensor_scalar(out=YY[:, 1, :], in0=v8_t[:], scalar1=dyp[:, 0:1],
                                    scalar2=ttcol(6, r), op0=Alu.mult, op1=Alu.add)

            # coord pipe: [128, 16] each for x and y
            def pipe(PPin, limit, tagp):
                F = 2 * T8
                vv = tabs.tile([128, F], f32, tag=tagp + "v")
                v2 = tabs.tile([128, F], f32, tag=tagp + "v2")
                xx = tabs.tile([128, F], f32, tag=tagp + "x")
                xi = tabs.tile([128, F], i32, tag=tagp + "i")
                xf = tabs.tile([128, F], f32, tag=tagp + "f")
                xfc = tabs.tile([128, F], f32, tag=tagp + "fc")
                lo = tabs.tile([128, F], f32, tag=tagp + "lo")
                mm = tabs.tile([128, F], f32, tag=tagp + "m")
                lx = tabs.tile([128, F], f32, tag=tagp + "l")
                hx = tabs.tile([128, F], f32, tag=tagp + "h")
                P = PPin[:].rearrange("p a t -> p (a t)")
                nc.vector.tensor_scalar(out=vv[:], in0=P, scalar1=-1.0, scalar2=None, op0=Alu.is_gt)
                nc.vector.tensor_scalar(out=v2[:], in0=P, scalar1=float(limit), scalar2=None, op0=Alu.is_lt)
                nc.vector.tensor_tensor(out=vv[:], in0=vv[:], in1=v2[:], op=Alu.mult)
                nc.vector.tensor_scalar(out=xx[:], in0=P, scalar1=0.0, scalar2=None, op0=Alu.max)
                nc.vector.tensor_scalar(out=xi[:], in0=xx[:], scalar1=0.5, scalar2=None, op0=Alu.subtract)
                nc.vector.tensor_copy(out=xf[:], in_=xi[:])
                nc.vector.tensor_scalar(out=xfc[:], in0=xf[:], scalar1=float(limit - 1),
                                        scalar2=None, op0=Alu.min)
                nc.vector.tensor_tensor(out=lo[:], in0=xx[:], in1=xfc[:], op=Alu.subtract)
                nc.vector.tensor_scalar(out=mm[:], in0=xfc[:], scalar1=float(limit - 1),
                                        scalar2=None, op0=Alu.is_lt)
                nc.vector.tensor_tensor(out=lx[:], in0=lo[:], in1=mm[:], op=Alu.mult)
                nc.vector.tensor_scalar(out=hx[:], in0=lx[:], scalar1=-1.0, scalar2=1.0,
                                        op0=Alu.mult, op1=Alu.add)
                return vv, xfc, lx, hx

            vx, xfc, lx, hx = pipe(XX, W, "px")
            vy, yfc, ly, hy = pipe(YY, H, "py")

            # views: x-quantities [128,(ix,t)] -> (iy, ix, t); y [128,(iy,t)] -> (iy, ix, t)
            def xv(tl):
                a = tl[:]
                return bass.AP(tensor=a.tensor, offset=a.offset,
                               ap=[list(a.ap[0]), [0, IY], [T8, IX], [1, T8]])

            def yv(tl):
                a = tl[:]
                return bass.AP(tensor=a.tensor, offset=a.offset,
                               ap=[list(a.ap[0]), [T8, IY], [0, IX], [1, T8]])

            SFREE = IY * IX * T8   # 32, col = iy*16 + ix*8 + t
            # offsets: o = (yfc*W + base) + xfc  -> int32
            yw = tabs.tile([128, IY * T8], f32, tag="yw")
            nc.vector.tensor_scalar(out=yw[:], in0=yfc[:], scalar1=float(W),
                                    scalar2=ttcol(8, r), op0=Alu.mult, op1=Alu.add)
            of = tabs.tile([128, SFREE], f32, tag="of")
            nc.vector.tensor_tensor(out=of[:], in0=yv(yw), in1=xv(xfc), op=Alu.add)
            O = tabs.tile([128, SFREE], i32, tag="O")
            nc.vector.tensor_copy(out=O[:], in_=of[:])

            # weights W4 [128, 4, 32]: corners TL BL TR BR
            q0 = tabs.tile([128, SFREE], f32, tag="q0")
            nc.vector.tensor_tensor(out=q0[:], in0=yv(vy), in1=xv(vx), op=Alu.mult)
            nc.vector.tensor_scalar(out=q0[:], in0=q0[:], scalar1=0.25, scalar2=None, op0=Alu.mult)
            hyq = tabs.tile([128, SFREE], f32, tag="hyq")
            lyq = tabs.tile([128, SFREE], f32, tag="lyq")
            nc.vector.tensor_tensor(out=hyq[:], in0=yv(hy), in1=q0[:], op=Alu.mult)
            nc.vector.tensor_tensor(out=lyq[:], in0=yv(ly), in1=q0[:], op=Alu.mult)
            W4 = tabs.tile([128, 4, SFREE], f32, tag="W4")
            nc.vector.tensor_tensor(out=W4[:, 0, :], in0=hyq[:], in1=xv(hx), op=Alu.mult)
            nc.vector.tensor_tensor(out=W4[:, 1, :], in0=lyq[:], in1=xv(hx), op=Alu.mult)
            nc.vector.tensor_tensor(out=W4[:, 2, :], in0=hyq[:], in1=xv(lx), op=Alu.mult)
            nc.vector.tensor_tensor(out=W4[:, 3, :], in0=lyq[:], in1=xv(lx), op=Alu.mult)

            # per tile: gather + combine
            for t in range(NT):
                g = gpool.tile([128, 4, 4 * C], gdt, tag="g")
                cols = [iy * 16 + ix * 8 + t for iy in range(2) for ix in range(2)]
                for si, col in enumerate(cols):
                    nc.gpsimd.indirect_dma_start(
                        out=g[:, si, :], out_offset=None, in_=feat4[:],
                        in_offset=bass.IndirectOffsetOnAxis(ap=O[:, col:col+1], axis=0))
                if USE_F32R:
                    # out[bins, C] = sum diag(w) @ G_sc ; fp32r 1 cyc/row at N=256
                    ps1 = pp_mm.tile([128, 2 * 128], f32, tag="ps1", space="PSUM")
                    nmm = 0
                    for si, col in enumerate(cols):
                        for ci in range(4):
                            dg = dpool.tile([128, 128], f32r, tag="dg")
                            nc.any.tensor_scalar(out=dg[:], in0=ident[:],
                                                 scalar1=W4[:, ci, col:col+1],
                                                 scalar2=None, op0=Alu.mult)
                            nc.tensor.matmul(ps1[:], lhsT=dg[:], rhs=g[:, si, ci*C:(ci+1)*C],
                                             start=(nmm == 0), stop=(nmm == 15))
                            nmm += 1
                    sb1 = spool.tile([128, 2 * 128], f32, tag="sb1")
                    nc.scalar.copy(sb1[:], ps1[:])
                    # transpose [bins, C] -> [C, bins]
                    psA = pp_tr.tile([128, 128], f32, tag="psA", space="PSUM")
                    psB = pp_tr.tile([128, 128], f32, tag="psB", space="PSUM")
                    nc.tensor.transpose(out=psA[:], in_=sb1[:, 0:128], identity=ident[:])
                    nc.tensor.transpose(out=psB[:], in_=sb1[:, 128:256], identity=ident[:])
                    st = spool.tile([128, 2, 128], f32, tag="st")
                    nc.scalar.copy(st[:, 0, :], psA[:])
                    nc.scalar.copy(st[:, 1, :], psB[:])
                else:
                    psA = pp_mm.tile([128, 128], f32, tag="psA", space="PSUM")
                    psB = pp_mm.tile([128, 128], f32, tag="psB", space="PSUM")
                    nmm = 0
                    for si, col in enumerate(cols):
                        for ci in range(4):
                            dg = dpool.tile([128, 128], f32, tag="dg")
                            nc.any.tensor_scalar(out=dg[:], in0=ident[:],
                                                 scalar1=W4[:, ci, col:col+1],
                                                 scalar2=None, op0=Alu.mult)
                            nc.tensor.matmul(psA[:], lhsT=g[:, si, ci*C:ci*C+128], rhs=dg[:],
                                             start=(nmm == 0), stop=(nmm == 15))
                            nc.tensor.matmul(psB[:], lhsT=g[:, si, ci*C+128:ci*C+256], rhs=dg[:],
                                             start=(nmm == 0), stop=(nmm == 15))
                            nmm += 1
                    st = spool.tile([128, 2, 128], f32, tag="st")
                    nc.scalar.copy(st[:, 0, :], psA[:])
                    nc.scalar.copy(st[:, 1, :], psB[:])
                nc.sync.dma_start(out=out_v[r, :, :, t, :], in_=st[:])

        if nrep > 1:
            with tc.For_i(0, nrep, 1):
                main_work()
        else:
            main_work()

    nc.finalize()
    return nc


def _get_nc():
    if "nc" not in _CACHE:
        _CACHE["nc"] = _build_nc()
    return _CACHE["nc"]


def run_sharded(input, rois, **spmd_kwargs):
    """Run on 8 cores; returns (full_output, BassKernelResults)."""
    from concourse.bass_utils import run_bass_kernel_spmd

    x = np.ascontiguousarray(np.asarray(input, dtype=np.float32))
    rr = np.ascontiguousarray(np.asarray(rois, dtype=np.float32))
    feat4 = _build_feat4(x)
    basis32, v8 = _host_constants()

    in_maps = []
    for c in range(NCORES):
        in_maps.append({
            "feat4": feat4,
            "rois": np.ascontiguousarray(rr[c*K:(c+1)*K]),
            "basis": basis32,
            "v8c": v8,
        })
    nc = _get_nc()
    res = run_bass_kernel_spmd(nc, in_maps, core_ids=list(range(NCORES)), **spmd_kwargs)
    outp = np.concatenate([res.results[c]["out"] for c in range(NCORES)], axis=0)
    return outp, res


def kernel(input, rois):
    out, _ = run_sharded(input, rois)
    return out



# revision 3
# speedup vs baseline: 1.2579x; 1.2579x over previous
"""BezierAlign Trainium2 kernel.

Full inputs -> full output. Shards the R=256 ROIs across 8 NeuronCores (32
ROIs/core); the feature map is replicated to every core in a "quad block"
layout (each block holds the 2x2 pixel footprint of a bilinear sample, fp16)
so one indirect-DMA descriptor fetches all 4 corners of one sample.

Per-core device program:
  1. Evaluate the 4 cubic Bezier curves per ROI on 32 partitions (roi-major),
     fold the +-0.25*bin sample offsets and the -0.5 align shift into shifted
     endpoint curves, PE-transpose them to pw-on-partition layout.
  2. Per ROI, compute sample coords / validity / bilinear weights / gather
     offsets for all 1024 bins x 4 samples with ~40 DVE ops (bins on
     partitions, f32 throughout; floor via round(x-0.5) into int32).
  3. Per 128-bin tile: 4 indirect gathers ([128,1024] fp16 each), then 32
     fp16 matmuls (lhsT = gathered corner block, rhs = diag(weight)) that
     accumulate the output directly transposed as [C-half, bins] in 2 PSUM
     tiles; copy to SBUF, DMA out.
"""

import numpy as np

# problem shapes (hardcoded per contract)
N, C, H, W = 2, 256, 160, 160
R = 256
OUT_H, OUT_W = 16, 64
SCALE = 0.25
NCORES = 8
K = R // NCORES          # 32 rois per core
NT = (OUT_H * OUT_W) // 128   # 8 tiles of 128 bins per roi
HW = H * W

_CACHE = {}


def _host_constants():
    f32 = np.float32
    u = (np.arange(OUT_W, dtype=f32) / f32(OUT_W)).astype(f32)
    mt = (f32(1.0) - u).astype(f32)
    basis = np.stack([mt**3, 3 * u * mt**2, 3 * u**2 * mt, u**3]).astype(f32)  # [4,64]
    basis32 = np.broadcast_to(basis.reshape(1, 4 * OUT_W), (K, 4 * OUT_W)).copy()
    p = np.arange(128)
    t = np.arange(NT)
    v8 = (((2 * t[None, :] + (p[:, None] >= 64)).astype(f32)) / f32(16.0)).astype(f32)
    return basis32, v8


def _build_feat4(x):
    """x [N, C, H, W] f32 -> fp16 [N*H*W, 4C]; block(n,y,x) = [f(y,x),
    f(y+1,x), f(y,x+1), f(y+1,x+1)] with out-of-image parts zeroed."""
    f = np.ascontiguousarray(x.transpose(0, 2, 3, 1)).astype(np.float16)  # [N,H,W,C]
    fy = np.zeros_like(f)
    fy[:, :-1] = f[:, 1:]
    a = np.concatenate([f, fy], axis=-1)                  # [N,H,W,2C]
    ax = np.zeros_like(a)
    ax[:, :, :-1] = a[:, :, 1:]
    feat4 = np.concatenate([a, ax], axis=-1)              # [N,H,W,4C]
    return np.ascontiguousarray(feat4.reshape(N * HW, 4 * C))


def _build_nc(nrep=1):
    from contextlib import ExitStack
    import concourse.bacc as bacc
    import concourse.bass as bass
    import concourse.tile as tile
    from concourse import mybir
    from concourse.masks import make_identity

    f32 = mybir.dt.float32
    f16 = mybir.dt.float16
    i32 = mybir.dt.int32
    Alu = mybir.AluOpType

    nc = bacc.Bacc(None, target_bir_lowering=False)

    feat4 = nc.dram_tensor("feat4", [N * HW, 4 * C], f16, kind="ExternalInput")
    rois = nc.dram_tensor("rois", [K, 17], f32, kind="ExternalInput")
    basis = nc.dram_tensor("basis", [K, 4 * OUT_W], f32, kind="ExternalInput")
    v8c = nc.dram_tensor("v8c", [128, NT], f32, kind="ExternalInput")
    out = nc.dram_tensor("out", [K, C, OUT_H, OUT_W], f32, kind="ExternalOutput")
    # [K, C, 1024] -> (k, h, p, t, b): c = h*128 + p, bin = t*128 + b
    out_v = out.rearrange("k (h p) (t c) w -> k p h t (c w)", h=2, c=2)

    with tile.TileContext(nc) as tc, ExitStack() as ctx:
        singles = ctx.enter_context(tc.tile_pool(name="singles", bufs=1))
        scratch = ctx.enter_context(tc.tile_pool(name="scratch", bufs=2))
        tabs = ctx.enter_context(tc.tile_pool(name="tabs", bufs=3))
        gpool = ctx.enter_context(tc.tile_pool(name="gpool", bufs=4))
        dpool = ctx.enter_context(tc.tile_pool(name="dpool", bufs=8))
        spool = ctx.enter_context(tc.tile_pool(name="spool", bufs=4))
        pp_t = ctx.enter_context(tc.tile_pool(name="pp_t", bufs=1, space="PSUM"))
        pp_mm = ctx.enter_context(tc.tile_pool(name="pp_mm", bufs=3, space="PSUM"))

        ident = singles.tile([128, 128], f32)
        make_identity(nc, ident[:])
        ident_h = singles.tile([128, 128], f16)
        nc.vector.tensor_copy(out=ident_h[:], in_=ident[:])
        v8_t = singles.tile([128, NT], f32)
        nc.sync.dma_start(out=v8_t[:], in_=v8c[:])
        r_t = singles.tile([K, 17], f32)
        nc.sync.dma_start(out=r_t[:], in_=rois[:])
        b_t = singles.tile([K, 4, OUT_W], f32)
        nc.sync.dma_start(out=b_t[:], in_=basis[:].rearrange("k (a u) -> k a u", a=4))

        # control points: px = rois[:, 1::2]*0.25, py = rois[:, 2::2]*0.25
        px = scratch.tile([K, 8], f32, tag="px")
        py = scratch.tile([K, 8], f32, tag="py")
        r_ap = r_t[:]
        px_src = bass.AP(tensor=r_ap.tensor, offset=r_ap.offset + 1, ap=[list(r_ap.ap[0]), [2, 8]])
        py_src = bass.AP(tensor=r_ap.tensor, offset=r_ap.offset + 2, ap=[list(r_ap.ap[0]), [2, 8]])
        nc.vector.tensor_scalar(out=px[:], in0=px_src, scalar1=SCALE, scalar2=None, op0=Alu.mult)
        nc.vector.tensor_scalar(out=py[:], in0=py_src, scalar1=SCALE, scalar2=None, op0=Alu.mult)

        # curves [K, 64]: cv = sum_a B[a] * p[a(+4)]
        def bezier(dst, ptile, o):
            acc = scratch.tile([K, OUT_W], f32, tag="bzacc")
            tmp = scratch.tile([K, OUT_W], f32, tag="bztmp")
            nc.vector.tensor_scalar(out=acc[:], in0=b_t[:, 0, :], scalar1=ptile[:, o:o+1],
                                    scalar2=None, op0=Alu.mult)
            for a in (1, 2, 3):
                nc.vector.tensor_scalar(out=tmp[:], in0=b_t[:, a, :], scalar1=ptile[:, o+a:o+a+1],
                                        scalar2=None, op0=Alu.mult)
                nc.vector.tensor_tensor(out=dst[:] if a == 3 else acc[:],
                                        in0=acc[:], in1=tmp[:], op=Alu.add)

        x0 = scratch.tile([K, OUT_W], f32, tag="x0"); bezier(x0, px, 0)
        x1 = scratch.tile([K, OUT_W], f32, tag="x1"); bezier(x1, px, 4)
        y0 = scratch.tile([K, OUT_W], f32, tag="y0"); bezier(y0, py, 0)
        y1 = scratch.tile([K, OUT_W], f32, tag="y1"); bezier(y1, py, 4)

        # roi_w/h -> bwq = roi_w*0.25/64, bhq = roi_h*0.25/16  [K,1]
        def quarter_bin(ptile, scale_imm, tag):
            d1 = scratch.tile([K, 1], f32, tag=tag + "d1")
            d2 = scratch.tile([K, 1], f32, tag=tag + "d2")
            dn = scratch.tile([K, 1], f32, tag=tag + "dn")
            q = scratch.tile([K, 1], f32, tag=tag)
            nc.vector.tensor_tensor(out=d1[:], in0=ptile[:, 0:1], in1=ptile[:, 3:4], op=Alu.subtract)
            nc.vector.tensor_scalar(out=dn[:], in0=d1[:], scalar1=-1.0, scalar2=None, op0=Alu.mult)
            nc.vector.tensor_tensor(out=d1[:], in0=d1[:], in1=dn[:], op=Alu.max)
            nc.vector.tensor_tensor(out=d2[:], in0=ptile[:, 4:5], in1=ptile[:, 7:8], op=Alu.subtract)
            nc.vector.tensor_scalar(out=dn[:], in0=d2[:], scalar1=-1.0, scalar2=None, op0=Alu.mult)
            nc.vector.tensor_tensor(out=d2[:], in0=d2[:], in1=dn[:], op=Alu.max)
            nc.vector.tensor_tensor(out=d1[:], in0=d1[:], in1=d2[:], op=Alu.max)
            nc.vector.tensor_scalar(out=q[:], in0=d1[:], scalar1=scale_imm, scalar2=None, op0=Alu.mult)
            return q

        bwq = quarter_bin(px, 0.25 / OUT_W, "bwq")
        bhq = quarter_bin(py, 0.25 / OUT_H, "bhq")

        # 9 shifted curves [K, 64]: order xm0 xm1 xp0 xp1 ym0 ym1 yp0 yp1 base
        curves = scratch.tile([K, 9, OUT_W], f32, tag="curves")
        spec = [(x0, bwq, Alu.subtract, 0), (x1, bwq, Alu.subtract, 1),
                (x0, bwq, Alu.add, 2), (x1, bwq, Alu.add, 3),
                (y0, bhq, Alu.subtract, 4), (y1, bhq, Alu.subtract, 5),
                (y0, bhq, Alu.add, 6), (y1, bhq, Alu.add, 7)]
        for cv, qq, op, idx in spec:
            nc.vector.tensor_scalar(out=curves[:, idx, :], in0=cv[:], scalar1=qq[:, 0:1],
                                    scalar2=0.5, op0=op, op1=Alu.subtract)
        # base = batch * HW broadcast along 64
        base_c = scratch.tile([K, 1], f32, tag="base_c")
        nc.vector.tensor_scalar(out=base_c[:], in0=r_t[:, 0:1], scalar1=float(HW),
                                scalar2=None, op0=Alu.mult)
        bc_ap = base_c[:]
        nc.vector.tensor_scalar(
            out=curves[:, 8, :],
            in0=bass.AP(tensor=bc_ap.tensor, offset=bc_ap.offset, ap=[list(bc_ap.ap[0]), [0, OUT_W]]),
            scalar1=0.0, scalar2=None, op0=Alu.add)

        # transpose to TT [128, 9, K]: TT[p, q, r] = curves[r, q, p % 64]
        TT = singles.tile([128, 9, K], f32)
        for q in range(9):
            ps = pp_t.tile([128, K], f32, tag="tps", space="PSUM")
            cdup = scratch.tile([K, 128], f32, tag="cdup")
            cin = curves[:, q, :]
            dup = bass.AP(tensor=cin.tensor, offset=cin.offset,
                          ap=[list(cin.ap[0]), [0, 2], list(cin.ap[-1])])
            nc.vector.tensor_copy(out=cdup[:], in_=dup)
            nc.tensor.transpose(out=ps[:], in_=cdup[:], identity=ident[:K, :K])
            nc.vector.tensor_copy(out=TT[:, q, :], in_=ps[:])

        def ttcol(q, r):
            return TT[:, q, r:r+1]

        IY, IX, T8 = 2, 2, NT

        def main_work():
         for r in range(K):
            # deltas [128,1]
            dxm = tabs.tile([128, 1], f32, tag="dxm")
            dxp = tabs.tile([128, 1], f32, tag="dxp")
            dym = tabs.tile([128, 1], f32, tag="dym")
            dyp = tabs.tile([128, 1], f32, tag="dyp")
            nc.vector.tensor_tensor(out=dxm[:], in0=ttcol(1, r), in1=ttcol(0, r), op=Alu.subtract)
            nc.vector.tensor_tensor(out=dxp[:], in0=ttcol(3, r), in1=ttcol(2, r), op=Alu.subtract)
            nc.vector.tensor_tensor(out=dym[:], in0=ttcol(5, r), in1=ttcol(4, r), op=Alu.subtract)
            nc.vector.tensor_tensor(out=dyp[:], in0=ttcol(7, r), in1=ttcol(6, r), op=Alu.subtract)

            # XX [128, 2(ix), 8(t)] = x0S + V8*dx ; YY [128, 2(iy), 8]
            XX = tabs.tile([128, IX, T8], f32, tag="XX")
            YY = tabs.tile([128, IY, T8], f32, tag="YY")
            nc.vector.tensor_scalar(out=XX[:, 0, :], in0=v8_t[:], scalar1=dxm[:, 0:1],
                                    scalar2=ttcol(0, r), op0=Alu.mult, op1=Alu.add)
            nc.vector.tensor_scalar(out=XX[:, 1, :], in0=v8_t[:], scalar1=dxp[:, 0:1],
                                    scalar2=ttcol(2, r), op0=Alu.mult, op1=Alu.add)
            nc.vector.tensor_scalar(out=YY[:, 0, :], in0=v8_t[:], scalar1=dym[:, 0:1],
                                    scalar2=ttcol(4, r), op0=Alu.mult, op1=Alu.add)
            nc.vector.tensor_scalar(out=YY[:, 1, :], in0=v8_t[:], scalar1=dyp[:, 0:1],
                                    scalar2=ttcol(6, r), op0=Alu.mult, op1=Alu.add)

            # coord pipe: [128, 16] each for x and y
            def pipe(PPin, limit, tagp):
                F = 2 * T8
                vv = tabs.tile([128, F], f32, tag=tagp + "v")
                v2 = tabs.tile([128, F], f32, tag=tagp + "v2")
                xx = tabs.tile([128, F], f32, tag=tagp + "x")
                xi = tabs.tile([128, F], i32, tag=tagp + "i")
                xf = tabs.tile([128, F], f32, tag=tagp + "f")
                xfc = tabs.tile([128, F], f32, tag=tagp + "fc")
                lo = tabs.tile([128, F], f32, tag=tagp + "lo")
                mm = tabs.tile([128, F], f32, tag=tagp + "m")
                lx = tabs.tile([128, F], f32, tag=tagp + "l")
                hx = tabs.tile([128, F], f32, tag=tagp + "h")
                P = PPin[:].rearrange("p a t -> p (a t)")
                nc.vector.tensor_scalar(out=vv[:], in0=P, scalar1=-1.0, scalar2=None, op0=Alu.is_gt)
                nc.vector.tensor_scalar(out=v2[:], in0=P, scalar1=float(limit), scalar2=None, op0=Alu.is_lt)
                nc.vector.tensor_tensor(out=vv[:], in0=vv[:], in1=v2[:], op=Alu.mult)
                nc.vector.tensor_scalar(out=xx[:], in0=P, scalar1=0.0, scalar2=None, op0=Alu.max)
                nc.vector.tensor_scalar(out=xi[:], in0=xx[:], scalar1=0.5, scalar2=None, op0=Alu.subtract)
                nc.vector.tensor_copy(out=xf[:], in_=xi[:])
                nc.vector.tensor_scalar(out=xfc[:], in0=xf[:], scalar1=float(limit - 1),
                                        scalar2=None, op0=Alu.min)
                nc.vector.tensor_tensor(out=lo[:], in0=xx[:], in1=xfc[:], op=Alu.subtract)
                nc.vector.tensor_scalar(out=mm[:], in0=xfc[:], scalar1=float(limit - 1),
                                        scalar2=None, op0=Alu.is_lt)
                nc.vector.tensor_tensor(out=lx[:], in0=lo[:], in1=mm[:], op=Alu.mult)
                nc.vector.tensor_scalar(out=hx[:], in0=lx[:], scalar1=-1.0, scalar2=1.0,
                                        op0=Alu.mult, op1=Alu.add)
                return vv, xfc, lx, hx

            vx, xfc, lx, hx = pipe(XX, W, "px")
            vy, yfc, ly, hy = pipe(YY, H, "py")

            # views: x-quantities [128,(ix,t)] -> (iy, ix, t); y [128,(iy,t)] -> (iy, ix, t)
            def xv(tl):
                a = tl[:]
                return bass.AP(tensor=a.tensor, offset=a.offset,
                               ap=[list(a.ap[0]), [0, IY], [T8, IX], [1, T8]])

            def yv(tl):
                a = tl[:]
                return bass.AP(tensor=a.tensor, offset=a.offset,
                               ap=[list(a.ap[0]), [T8, IY], [0, IX], [1, T8]])

            SFREE = IY * IX * T8   # 32, col = iy*16 + ix*8 + t
            # offsets: o = (yfc*W + base) + xfc  -> int32
            yw = tabs.tile([128, IY * T8], f32, tag="yw")
            nc.vector.tensor_scalar(out=yw[:], in0=yfc[:], scalar1=float(W),
                                    scalar2=ttcol(8, r), op0=Alu.mult, op1=Alu.add)
            of = tabs.tile([128, SFREE], f32, tag="of")
            nc.vector.tensor_tensor(out=of[:], in0=yv(yw), in1=xv(xfc), op=Alu.add)
            O = tabs.tile([128, SFREE], i32, tag="O")
            nc.vector.tensor_copy(out=O[:], in_=of[:])

            # weights W4 [128, 4, 32]: corners TL BL TR BR
            q0 = tabs.tile([128, SFREE], f32, tag="q0")
            nc.vector.tensor_tensor(out=q0[:], in0=yv(vy), in1=xv(vx), op=Alu.mult)
            nc.vector.tensor_scalar(out=q0[:], in0=q0[:], scalar1=0.25, scalar2=None, op0=Alu.mult)
            hyq = tabs.tile([128, SFREE], f32, tag="hyq")
            lyq = tabs.tile([128, SFREE], f32, tag="lyq")
            nc.vector.tensor_tensor(out=hyq[:], in0=yv(hy), in1=q0[:], op=Alu.mult)
            nc.vector.tensor_tensor(out=lyq[:], in0=yv(ly), in1=q0[:], op=Alu.mult)
            W4 = tabs.tile([128, 4, SFREE], f32, tag="W4")
            nc.vector.tensor_tensor(out=W4[:, 0, :], in0=hyq[:], in1=xv(hx), op=Alu.mult)
            nc.vector.tensor_tensor(out=W4[:, 1, :], in0=lyq[:], in1=xv(hx), op=Alu.mult)
            nc.vector.tensor_tensor(out=W4[:, 2, :], in0=hyq[:], in1=xv(lx), op=Alu.mult)
            nc.vector.tensor_tensor(out=W4[:, 3, :], in0=lyq[:], in1=xv(lx), op=Alu.mult)

            # per tile: gather + combine directly into [C-half, bins] PSUM
            for t in range(NT):
                g = gpool.tile([128, 4, 4 * C], f16, tag="g")
                cols = [iy * 16 + ix * 8 + t for iy in range(2) for ix in range(2)]
                for si, col in enumerate(cols):
                    nc.gpsimd.indirect_dma_start(
                        out=g[:, si, :], out_offset=None, in_=feat4[:],
                        in_offset=bass.IndirectOffsetOnAxis(ap=O[:, col:col+1], axis=0))
                psA = pp_mm.tile([128, 128], f32, tag="psA", space="PSUM")
                psB = pp_mm.tile([128, 128], f32, tag="psB", space="PSUM")
                nmm = 0
                for si, col in enumerate(cols):
                    for ci in range(4):
                        dg = dpool.tile([128, 128], f16, tag="dg")
                        # diag(w): per-partition scale of the identity
                        if ci == 3:
                            nc.scalar.mul(out=dg[:], in_=ident_h[:],
                                          mul=W4[:, ci, col:col+1])
                        else:
                            nc.vector.tensor_scalar(out=dg[:], in0=ident_h[:],
                                                    scalar1=W4[:, ci, col:col+1],
                                                    scalar2=None, op0=Alu.mult)
                        nc.tensor.matmul(psA[:], lhsT=g[:, si, ci*C:ci*C+128], rhs=dg[:],
                                         start=(nmm == 0), stop=(nmm == 15))
                        nc.tensor.matmul(psB[:], lhsT=g[:, si, ci*C+128:ci*C+256], rhs=dg[:],
                                         start=(nmm == 0), stop=(nmm == 15))
                        nmm += 1
                st = spool.tile([128, 2, 128], f32, tag="st")
                nc.scalar.copy(st[:, 0, :], psA[:])
                nc.scalar.copy(st[:, 1, :], psB[:])
                nc.sync.dma_start(out=out_v[r, :, :, t, :], in_=st[:])

        if nrep > 1:
            with tc.For_i(0, nrep, 1):
                main_work()
        else:
            main_work()

    nc.finalize()
    return nc


def _get_nc():
    if "nc" not in _CACHE:
        _CACHE["nc"] = _build_nc()
    return _CACHE["nc"]


def run_sharded(input, rois, **spmd_kwargs):
    """Run on 8 cores; returns (full_output, BassKernelResults)."""
    from concourse.bass_utils import run_bass_kernel_spmd

    x = np.ascontiguousarray(np.asarray(input, dtype=np.float32))
    rr = np.ascontiguousarray(np.asarray(rois, dtype=np.float32))
    feat4 = _build_feat4(x)
    basis32, v8 = _host_constants()

    in_maps = []
    for c in range(NCORES):
        in_maps.append({
            "feat4": feat4,
            "rois": np.ascontiguousarray(rr[c*K:(c+1)*K]),
            "basis": basis32,
            "v8c": v8,
        })
    nc = _get_nc()
    res = run_bass_kernel_spmd(nc, in_maps, core_ids=list(range(NCORES)), **spmd_kwargs)
    outp = np.concatenate([res.results[c]["out"] for c in range(NCORES)], axis=0)
    return outp, res


def kernel(input, rois):
    out, _ = run_sharded(input, rois)
    return out


# revision 4
# speedup vs baseline: 2.0417x; 1.6231x over previous
"""BezierAlign Trainium2 kernel.

Full inputs -> full output. Shards the R=256 ROIs across 8 NeuronCores (32
ROIs/core); the feature map is replicated to every core in a "window block"
layout: block(n,y,x) holds the 4-wide x 2-tall fp16 pixel window
[f(y,x..x+3) x f(y..y+1)] = 8C values = 4KB, so ONE indirect-DMA descriptor
fetches the footprint of BOTH x-samples of a bin row (max bin width 2.5 px
=> x_low spread <= 2). Indirect gathers are descriptor-rate-bound (~8.6 ns
per 128-desc row), so halving descriptor count halves gather time.

Per-core device program:
  1. Evaluate the 4 cubic Bezier curves per ROI on 32 partitions (roi-major),
     fold the +-0.25*bin sample offsets and the -0.5 align shift into shifted
     endpoint curves, PE-transpose them to pw-on-partition layout.
  2. Per ROI, compute sample coords / validity / per-window weights W8
     (4 x-positions x 2 y-rows, with dx-selection masks merging the two
     x-samples) / gather offsets, bins on partitions, f32 DVE throughout.
  3. Per 128-bin tile: 2 indirect gathers ([128, 8C] fp16, one per y-sample
     row), then 32 fp16 matmuls (lhsT = gathered pixel-column block,
     rhs = diag(weight)) accumulating the output directly transposed as
     [C-half, bins] in 2 PSUM tiles; copy to SBUF, DMA out.
"""

import numpy as np

# problem shapes (hardcoded per contract)
N, C, H, W = 2, 256, 160, 160
R = 256
OUT_H, OUT_W = 16, 64
SCALE = 0.25
NCORES = 8
K = R // NCORES          # 32 rois per core
NT = (OUT_H * OUT_W) // 128   # 8 tiles of 128 bins per roi
HW = H * W
NB = 8                   # blocks per window: 4 dx * 2 y

_CACHE = {}


def _host_constants():
    f32 = np.float32
    u = (np.arange(OUT_W, dtype=f32) / f32(OUT_W)).astype(f32)
    mt = (f32(1.0) - u).astype(f32)
    basis = np.stack([mt**3, 3 * u * mt**2, 3 * u**2 * mt, u**3]).astype(f32)  # [4,64]
    basis32 = np.broadcast_to(basis.reshape(1, 4 * OUT_W), (K, 4 * OUT_W)).copy()
    p = np.arange(128)
    t = np.arange(NT)
    v8 = (((2 * t[None, :] + (p[:, None] >= 64)).astype(f32)) / f32(16.0)).astype(f32)
    return basis32, v8


def _build_feat8(x):
    """x [N, C, H, W] f32 -> fp16 [N*H*W, 8C]; block(n,y,x) =
    [f(y,x), f(y+1,x), f(y,x+1), f(y+1,x+1), ..., f(y,x+3), f(y+1,x+3)]
    with out-of-image parts zeroed."""
    f = np.ascontiguousarray(x.transpose(0, 2, 3, 1)).astype(np.float16)  # [N,H,W,C]
    fy = np.zeros_like(f)
    fy[:, :-1] = f[:, 1:]
    a = np.concatenate([f, fy], axis=-1)                  # [N,H,W,2C] y-pair
    parts = [a]
    for dx in (1, 2, 3):
        ax = np.zeros_like(a)
        ax[:, :, :-dx] = a[:, :, dx:]
        parts.append(ax)
    feat8 = np.concatenate(parts, axis=-1)                # [N,H,W,8C]
    return np.ascontiguousarray(feat8.reshape(N * HW, NB * C))


def _build_nc(nrep=1):
    from contextlib import ExitStack
    import concourse.bacc as bacc
    import concourse.bass as bass
    import concourse.tile as tile
    from concourse import mybir
    from concourse.masks import make_identity

    f32 = mybir.dt.float32
    f16 = mybir.dt.float16
    i32 = mybir.dt.int32
    Alu = mybir.AluOpType

    nc = bacc.Bacc(None, target_bir_lowering=False)

    feat8 = nc.dram_tensor("feat8", [N * HW, NB * C], f16, kind="ExternalInput")
    rois = nc.dram_tensor("rois", [K, 17], f32, kind="ExternalInput")
    basis = nc.dram_tensor("basis", [K, 4 * OUT_W], f32, kind="ExternalInput")
    v8c = nc.dram_tensor("v8c", [128, NT], f32, kind="ExternalInput")
    out = nc.dram_tensor("out", [K, C, OUT_H, OUT_W], f32, kind="ExternalOutput")
    # [K, C, 1024] -> (k, h, p, t, b): c = h*128 + p, bin = t*128 + b
    out_v = out.rearrange("k (h p) (t c) w -> k p h t (c w)", h=2, c=2)

    with tile.TileContext(nc) as tc, ExitStack() as ctx:
        singles = ctx.enter_context(tc.tile_pool(name="singles", bufs=1))
        scratch = ctx.enter_context(tc.tile_pool(name="scratch", bufs=2))
        tabs = ctx.enter_context(tc.tile_pool(name="tabs", bufs=3))
        gpool = ctx.enter_context(tc.tile_pool(name="gpool", bufs=4))
        dpool = ctx.enter_context(tc.tile_pool(name="dpool", bufs=8))
        spool = ctx.enter_context(tc.tile_pool(name="spool", bufs=4))
        pp_t = ctx.enter_context(tc.tile_pool(name="pp_t", bufs=1, space="PSUM"))
        pp_mm = ctx.enter_context(tc.tile_pool(name="pp_mm", bufs=3, space="PSUM"))

        ident = singles.tile([128, 128], f32)
        make_identity(nc, ident[:])
        ident_h = singles.tile([128, 128], f16)
        nc.vector.tensor_copy(out=ident_h[:], in_=ident[:])
        v8_t = singles.tile([128, NT], f32)
        nc.sync.dma_start(out=v8_t[:], in_=v8c[:])
        r_t = singles.tile([K, 17], f32)
        nc.sync.dma_start(out=r_t[:], in_=rois[:])
        b_t = singles.tile([K, 4, OUT_W], f32)
        nc.sync.dma_start(out=b_t[:], in_=basis[:].rearrange("k (a u) -> k a u", a=4))

        # control points: px = rois[:, 1::2]*0.25, py = rois[:, 2::2]*0.25
        px = scratch.tile([K, 8], f32, tag="px")
        py = scratch.tile([K, 8], f32, tag="py")
        r_ap = r_t[:]
        px_src = bass.AP(tensor=r_ap.tensor, offset=r_ap.offset + 1, ap=[list(r_ap.ap[0]), [2, 8]])
        py_src = bass.AP(tensor=r_ap.tensor, offset=r_ap.offset + 2, ap=[list(r_ap.ap[0]), [2, 8]])
        nc.vector.tensor_scalar(out=px[:], in0=px_src, scalar1=SCALE, scalar2=None, op0=Alu.mult)
        nc.vector.tensor_scalar(out=py[:], in0=py_src, scalar1=SCALE, scalar2=None, op0=Alu.mult)

        # curves [K, 64]: cv = sum_a B[a] * p[a(+4)]
        def bezier(dst, ptile, o):
            acc = scratch.tile([K, OUT_W], f32, tag="bzacc")
            tmp = scratch.tile([K, OUT_W], f32, tag="bztmp")
            nc.vector.tensor_scalar(out=acc[:], in0=b_t[:, 0, :], scalar1=ptile[:, o:o+1],
                                    scalar2=None, op0=Alu.mult)
            for a in (1, 2, 3):
                nc.vector.tensor_scalar(out=tmp[:], in0=b_t[:, a, :], scalar1=ptile[:, o+a:o+a+1],
                                        scalar2=None, op0=Alu.mult)
                nc.vector.tensor_tensor(out=dst[:] if a == 3 else acc[:],
                                        in0=acc[:], in1=tmp[:], op=Alu.add)

        x0 = scratch.tile([K, OUT_W], f32, tag="x0"); bezier(x0, px, 0)
        x1 = scratch.tile([K, OUT_W], f32, tag="x1"); bezier(x1, px, 4)
        y0 = scratch.tile([K, OUT_W], f32, tag="y0"); bezier(y0, py, 0)
        y1 = scratch.tile([K, OUT_W], f32, tag="y1"); bezier(y1, py, 4)

        # roi_w/h -> bwq = roi_w*0.25/64, bhq = roi_h*0.25/16  [K,1]
        def quarter_bin(ptile, scale_imm, tag):
            d1 = scratch.tile([K, 1], f32, tag=tag + "d1")
            d2 = scratch.tile([K, 1], f32, tag=tag + "d2")
            dn = scratch.tile([K, 1], f32, tag=tag + "dn")
            q = scratch.tile([K, 1], f32, tag=tag)
            nc.vector.tensor_tensor(out=d1[:], in0=ptile[:, 0:1], in1=ptile[:, 3:4], op=Alu.subtract)
            nc.vector.tensor_scalar(out=dn[:], in0=d1[:], scalar1=-1.0, scalar2=None, op0=Alu.mult)
            nc.vector.tensor_tensor(out=d1[:], in0=d1[:], in1=dn[:], op=Alu.max)
            nc.vector.tensor_tensor(out=d2[:], in0=ptile[:, 4:5], in1=ptile[:, 7:8], op=Alu.subtract)
            nc.vector.tensor_scalar(out=dn[:], in0=d2[:], scalar1=-1.0, scalar2=None, op0=Alu.mult)
            nc.vector.tensor_tensor(out=d2[:], in0=d2[:], in1=dn[:], op=Alu.max)
            nc.vector.tensor_tensor(out=d1[:], in0=d1[:], in1=d2[:], op=Alu.max)
            nc.vector.tensor_scalar(out=q[:], in0=d1[:], scalar1=scale_imm, scalar2=None, op0=Alu.mult)
            return q

        bwq = quarter_bin(px, 0.25 / OUT_W, "bwq")
        bhq = quarter_bin(py, 0.25 / OUT_H, "bhq")

        # 9 shifted curves [K, 64]: order xm0 xm1 xp0 xp1 ym0 ym1 yp0 yp1 base
        curves = scratch.tile([K, 9, OUT_W], f32, tag="curves")
        spec = [(x0, bwq, Alu.subtract, 0), (x1, bwq, Alu.subtract, 1),
                (x0, bwq, Alu.add, 2), (x1, bwq, Alu.add, 3),
                (y0, bhq, Alu.subtract, 4), (y1, bhq, Alu.subtract, 5),
                (y0, bhq, Alu.add, 6), (y1, bhq, Alu.add, 7)]
        for cv, qq, op, idx in spec:
            nc.vector.tensor_scalar(out=curves[:, idx, :], in0=cv[:], scalar1=qq[:, 0:1],
                                    scalar2=0.5, op0=op, op1=Alu.subtract)
        # base = batch * HW broadcast along 64
        base_c = scratch.tile([K, 1], f32, tag="base_c")
        nc.vector.tensor_scalar(out=base_c[:], in0=r_t[:, 0:1], scalar1=float(HW),
                                scalar2=None, op0=Alu.mult)
        bc_ap = base_c[:]
        nc.vector.tensor_scalar(
            out=curves[:, 8, :],
            in0=bass.AP(tensor=bc_ap.tensor, offset=bc_ap.offset, ap=[list(bc_ap.ap[0]), [0, OUT_W]]),
            scalar1=0.0, scalar2=None, op0=Alu.add)

        # transpose to TT [128, 9, K]: TT[p, q, r] = curves[r, q, p % 64]
        TT = singles.tile([128, 9, K], f32)
        for q in range(9):
            ps = pp_t.tile([128, K], f32, tag="tps", space="PSUM")
            cdup = scratch.tile([K, 128], f32, tag="cdup")
            cin = curves[:, q, :]
            dup = bass.AP(tensor=cin.tensor, offset=cin.offset,
                          ap=[list(cin.ap[0]), [0, 2], list(cin.ap[-1])])
            nc.vector.tensor_copy(out=cdup[:], in_=dup)
            nc.tensor.transpose(out=ps[:], in_=cdup[:], identity=ident[:K, :K])
            nc.vector.tensor_copy(out=TT[:, q, :], in_=ps[:])

        def ttcol(q, r):
            return TT[:, q, r:r+1]

        IY, IX, T8 = 2, 2, NT

        def main_work():
         for r in range(K):
            # deltas [128,1]
            dxm = tabs.tile([128, 1], f32, tag="dxm")
            dxp = tabs.tile([128, 1], f32, tag="dxp")
            dym = tabs.tile([128, 1], f32, tag="dym")
            dyp = tabs.tile([128, 1], f32, tag="dyp")
            nc.vector.tensor_tensor(out=dxm[:], in0=ttcol(1, r), in1=ttcol(0, r), op=Alu.subtract)
            nc.vector.tensor_tensor(out=dxp[:], in0=ttcol(3, r), in1=ttcol(2, r), op=Alu.subtract)
            nc.vector.tensor_tensor(out=dym[:], in0=ttcol(5, r), in1=ttcol(4, r), op=Alu.subtract)
            nc.vector.tensor_tensor(out=dyp[:], in0=ttcol(7, r), in1=ttcol(6, r), op=Alu.subtract)

            # XX [128, 2(ix), 8(t)] = x0S + V8*dx ; YY [128, 2(iy), 8]
            XX = tabs.tile([128, IX, T8], f32, tag="XX")
            YY = tabs.tile([128, IY, T8], f32, tag="YY")
            nc.vector.tensor_scalar(out=XX[:, 0, :], in0=v8_t[:], scalar1=dxm[:, 0:1],
                                    scalar2=ttcol(0, r), op0=Alu.mult, op1=Alu.add)
            nc.vector.tensor_scalar(out=XX[:, 1, :], in0=v8_t[:], scalar1=dxp[:, 0:1],
                                    scalar2=ttcol(2, r), op0=Alu.mult, op1=Alu.add)
            nc.vector.tensor_scalar(out=YY[:, 0, :], in0=v8_t[:], scalar1=dym[:, 0:1],
                                    scalar2=ttcol(4, r), op0=Alu.mult, op1=Alu.add)
            nc.vector.tensor_scalar(out=YY[:, 1, :], in0=v8_t[:], scalar1=dyp[:, 0:1],
                                    scalar2=ttcol(6, r), op0=Alu.mult, op1=Alu.add)

            # coord pipe: [128, 16] each for x and y
            def pipe(PPin, limit, tagp):
                F = 2 * T8
                vv = tabs.tile([128, F], f32, tag=tagp + "v")
                v2 = tabs.tile([128, F], f32, tag=tagp + "v2")
                xx = tabs.tile([128, F], f32, tag=tagp + "x")
                xi = tabs.tile([128, F], i32, tag=tagp + "i")
                xf = tabs.tile([128, F], f32, tag=tagp + "f")
                xfc = tabs.tile([128, F], f32, tag=tagp + "fc")
                lo = tabs.tile([128, F], f32, tag=tagp + "lo")
                mm = tabs.tile([128, F], f32, tag=tagp + "m")
                lx = tabs.tile([128, F], f32, tag=tagp + "l")
                hx = tabs.tile([128, F], f32, tag=tagp + "h")
                P = PPin[:].rearrange("p a t -> p (a t)")
                nc.vector.tensor_scalar(out=vv[:], in0=P, scalar1=-1.0, scalar2=None, op0=Alu.is_gt)
                nc.vector.tensor_scalar(out=v2[:], in0=P, scalar1=float(limit), scalar2=None, op0=Alu.is_lt)
                nc.vector.tensor_tensor(out=vv[:], in0=vv[:], in1=v2[:], op=Alu.mult)
                nc.vector.tensor_scalar(out=xx[:], in0=P, scalar1=0.0, scalar2=None, op0=Alu.max)
                nc.vector.tensor_scalar(out=xi[:], in0=xx[:], scalar1=0.5, scalar2=None, op0=Alu.subtract)
                nc.vector.tensor_copy(out=xf[:], in_=xi[:])
                nc.vector.tensor_scalar(out=xfc[:], in0=xf[:], scalar1=float(limit - 1),
                                        scalar2=None, op0=Alu.min)
                nc.vector.tensor_tensor(out=lo[:], in0=xx[:], in1=xfc[:], op=Alu.subtract)
                nc.vector.tensor_scalar(out=mm[:], in0=xfc[:], scalar1=float(limit - 1),
                                        scalar2=None, op0=Alu.is_lt)
                nc.vector.tensor_tensor(out=lx[:], in0=lo[:], in1=mm[:], op=Alu.mult)
                nc.vector.tensor_scalar(out=hx[:], in0=lx[:], scalar1=-1.0, scalar2=1.0,
                                        op0=Alu.mult, op1=Alu.add)
                return vv, xfc, lx, hx

            vx, xfc, lx, hx = pipe(XX, W, "px")
            vy, yfc, ly, hy = pipe(YY, H, "py")

            # y-quantity views [128,(iy,t)] -> same; broadcast-x views for the
            # [128, t]-shaped x planes across iy
            def bx(tl, o=0):
                a = tl[:]
                return bass.AP(tensor=a.tensor, offset=a.offset + o,
                               ap=[list(a.ap[0]), [0, IY], [1, T8]])

            SFREE = IY * T8   # 16, col = iy*8 + t
            # per-sample x validity fold: hxv = hx*vx, lxv = lx*vx  [128, 16 (ix,t)]
            hxv = tabs.tile([128, IX * T8], f32, tag="hxv")
            lxv = tabs.tile([128, IX * T8], f32, tag="lxv")
            nc.vector.tensor_tensor(out=hxv[:], in0=hx[:], in1=vx[:], op=Alu.mult)
            nc.vector.tensor_tensor(out=lxv[:], in0=lx[:], in1=vx[:], op=Alu.mult)

            # window-position indicators: d1 = xfc1 - xfc0 in {0,1,2} [128, 8]
            d1 = tabs.tile([128, T8], f32, tag="d1")
            nc.vector.tensor_tensor(out=d1[:], in0=xfc[:, T8:2*T8], in1=xfc[:, 0:T8],
                                    op=Alu.subtract)
            i0 = tabs.tile([128, T8], f32, tag="i0")
            i1 = tabs.tile([128, T8], f32, tag="i1")
            i2 = tabs.tile([128, T8], f32, tag="i2")
            nc.vector.tensor_scalar(out=i0[:], in0=d1[:], scalar1=0.5, scalar2=None, op0=Alu.is_lt)
            nc.vector.tensor_scalar(out=i2[:], in0=d1[:], scalar1=1.5, scalar2=None, op0=Alu.is_gt)
            nc.vector.tensor_tensor(out=i1[:], in0=i0[:], in1=i2[:], op=Alu.add)
            nc.vector.tensor_scalar(out=i1[:], in0=i1[:], scalar1=-1.0, scalar2=1.0,
                                    op0=Alu.mult, op1=Alu.add)

            # x-weight planes wx[4] [128, 8]: merged contributions of the two
            # x-samples onto the 4 window columns
            # wx0 = hxv0 + i0*hxv1 ; wx1 = lxv0 + i0*lxv1 + i1*hxv1
            # wx2 = i1*lxv1 + i2*hxv1 ; wx3 = i2*lxv1
            WXT = tabs.tile([128, 4, T8], f32, tag="WXT")
            tmpa = tabs.tile([128, T8], f32, tag="tmpa")
            h1 = hxv[:, T8:2*T8]
            l1 = lxv[:, T8:2*T8]
            nc.vector.tensor_tensor(out=tmpa[:], in0=i0[:], in1=h1, op=Alu.mult)
            nc.vector.tensor_tensor(out=WXT[:, 0, :], in0=hxv[:, 0:T8], in1=tmpa[:], op=Alu.add)
            nc.vector.tensor_tensor(out=tmpa[:], in0=i0[:], in1=l1, op=Alu.mult)
            nc.vector.tensor_tensor(out=WXT[:, 1, :], in0=lxv[:, 0:T8], in1=tmpa[:], op=Alu.add)
            nc.vector.tensor_tensor(out=tmpa[:], in0=i1[:], in1=h1, op=Alu.mult)
            nc.vector.tensor_tensor(out=WXT[:, 1, :], in0=WXT[:, 1, :], in1=tmpa[:], op=Alu.add)
            nc.vector.tensor_tensor(out=tmpa[:], in0=i1[:], in1=l1, op=Alu.mult)
            nc.vector.tensor_tensor(out=WXT[:, 2, :], in0=tmpa[:], in1=tmpa[:], op=Alu.max)
            nc.vector.tensor_tensor(out=tmpa[:], in0=i2[:], in1=h1, op=Alu.mult)
            nc.vector.tensor_tensor(out=WXT[:, 2, :], in0=WXT[:, 2, :], in1=tmpa[:], op=Alu.add)
            nc.vector.tensor_tensor(out=WXT[:, 3, :], in0=i2[:], in1=l1, op=Alu.mult)

            # offsets: o = (yfc*W + base) + xfc0  [128, 16 (iy,t)] -> int32
            yw = tabs.tile([128, IY * T8], f32, tag="yw")
            nc.vector.tensor_scalar(out=yw[:], in0=yfc[:], scalar1=float(W),
                                    scalar2=ttcol(8, r), op0=Alu.mult, op1=Alu.add)
            of = tabs.tile([128, SFREE], f32, tag="of")
            nc.vector.tensor_tensor(out=of[:], in0=yw[:], in1=bx(xfc), op=Alu.add)
            O = tabs.tile([128, SFREE], i32, tag="O")
            nc.vector.tensor_copy(out=O[:], in_=of[:])

            # y weights folded with valid & 1/4: hyq = hy*vy*0.25, lyq likewise
            q0 = tabs.tile([128, SFREE], f32, tag="q0")
            nc.vector.tensor_scalar(out=q0[:], in0=vy[:], scalar1=0.25, scalar2=None, op0=Alu.mult)
            hyq = tabs.tile([128, SFREE], f32, tag="hyq")
            lyq = tabs.tile([128, SFREE], f32, tag="lyq")
            nc.vector.tensor_tensor(out=hyq[:], in0=hy[:], in1=q0[:], op=Alu.mult)
            nc.vector.tensor_tensor(out=lyq[:], in0=ly[:], in1=q0[:], op=Alu.mult)

            # W8 [128, 8 (2dx+ylh), 16 (iy,t)]: wx[dx] x (hyq, lyq)
            W8 = tabs.tile([128, NB, SFREE], f32, tag="W8")
            for dx in range(4):
                wx_b = bass.AP(tensor=WXT[:].tensor, offset=WXT[:].offset + dx * T8,
                               ap=[list(WXT[:].ap[0]), [0, IY], [1, T8]])
                nc.vector.tensor_tensor(out=W8[:, 2*dx, :], in0=hyq[:], in1=wx_b, op=Alu.mult)
                nc.vector.tensor_tensor(out=W8[:, 2*dx+1, :], in0=lyq[:], in1=wx_b, op=Alu.mult)

            # per tile: gather + combine directly into [C-half, bins] PSUM
            for t in range(NT):
                g = gpool.tile([128, IY, NB * C], f16, tag="g")
                for iy in range(IY):
                    col = iy * T8 + t
                    nc.gpsimd.indirect_dma_start(
                        out=g[:, iy, :], out_offset=None, in_=feat8[:],
                        in_offset=bass.IndirectOffsetOnAxis(ap=O[:, col:col+1], axis=0))
                psA = pp_mm.tile([128, 128], f32, tag="psA", space="PSUM")
                psB = pp_mm.tile([128, 128], f32, tag="psB", space="PSUM")
                nmm = 0
                for iy in range(IY):
                    col = iy * T8 + t
                    for blk in range(NB):
                        dg = dpool.tile([128, 128], f16, tag="dg")
                        # diag(w): per-partition scale of the identity
                        if blk in (3, 7):
                            nc.scalar.mul(out=dg[:], in_=ident_h[:],
                                          mul=W8[:, blk, col:col+1])
                        else:
                            nc.vector.tensor_scalar(out=dg[:], in0=ident_h[:],
                                                    scalar1=W8[:, blk, col:col+1],
                                                    scalar2=None, op0=Alu.mult)
                        nc.tensor.matmul(psA[:], lhsT=g[:, iy, blk*C:blk*C+128], rhs=dg[:],
                                         start=(nmm == 0), stop=(nmm == 15))
                        nc.tensor.matmul(psB[:], lhsT=g[:, iy, blk*C+128:blk*C+256], rhs=dg[:],
                                         start=(nmm == 0), stop=(nmm == 15))
                        nmm += 1
                st = spool.tile([128, 2, 128], f32, tag="st")
                nc.scalar.copy(st[:, 0, :], psA[:])
                nc.scalar.copy(st[:, 1, :], psB[:])
                nc.sync.dma_start(out=out_v[r, :, :, t, :], in_=st[:])

        if nrep > 1:
            with tc.For_i(0, nrep, 1):
                main_work()
        else:
            main_work()

    nc.finalize()
    return nc


def _get_nc():
    if "nc" not in _CACHE:
        _CACHE["nc"] = _build_nc()
    return _CACHE["nc"]


def run_sharded(input, rois, **spmd_kwargs):
    """Run on 8 cores; returns (full_output, BassKernelResults)."""
    from concourse.bass_utils import run_bass_kernel_spmd

    x = np.ascontiguousarray(np.asarray(input, dtype=np.float32))
    rr = np.ascontiguousarray(np.asarray(rois, dtype=np.float32))
    feat8 = _build_feat8(x)
    basis32, v8 = _host_constants()

    in_maps = []
    for c in range(NCORES):
        in_maps.append({
            "feat8": feat8,
            "rois": np.ascontiguousarray(rr[c*K:(c+1)*K]),
            "basis": basis32,
            "v8c": v8,
        })
    nc = _get_nc()
    res = run_bass_kernel_spmd(nc, in_maps, core_ids=list(range(NCORES)), **spmd_kwargs)
    outp = np.concatenate([res.results[c]["out"] for c in range(NCORES)], axis=0)
    return outp, res


def kernel(input, rois):
    out, _ = run_sharded(input, rois)
    return out
